# revision 1
# baseline (speedup 1.0000x reference)
"""Trainium2 Bass kernel: 7x7 valid 2D cross-correlation on a 6144x6144 fp32
image, + scalar bias. Output 6138x6138 fp32.

Strategy (t32 scheme)
---------------------
Row-band sharding across 8 NeuronCores: core c computes output rows
[c*768, c*768+768) for all 6138 output columns. Inputs stream as fp16
(rel err ~3e-4, well inside the 2e-2 gate); output is stored fp16 and
upcast on host.

Per core the conv runs as 16 CONCURRENT 32x32-tile banded matmuls on the
PE array (4 row-groups x 4 col-groups, tile_position packing):

  - output rows are split into 26-row blocks; block k lives in SBUF
    partition group (k mod 4) of a 128-partition window tile holding
    input rows [104w + 26b, 104w + 26b + 32) (26 outputs + 6 halo).
  - stationary for tap j, block-group b: A[k, m] = W[k - m, j]
    (0 <= k-m < 7, m < 26), a [32, 26] band at partitions [32b, 32b+32).
  - col-group g of the array processes output-column quarter
    [2048c + 512g, +512); its matmul streams the same partitions with a
    j-shifted AP and writes PSUM partitions [32g, 32g+26) of bank b.
  - 7 taps accumulate per bank; 4 banks (blocks) per (window, chunk)
    round, double-buffered against eviction (8 PSUM banks total).

Each cycle the array retires 16 tiles x 26 outputs x 7-tap band MACs
(~2900 useful MACs/cycle vs ~850 for a full-width 122-row band matmul).
Eviction adds bias via DVE tensor_scalar_add (fp32 PSUM -> fp16 SBUF);
stores write y as [768, 6144] fp16 (padded; host trims to 6138).
"""

import os

import numpy as np

import concourse.tile as tile
from concourse import bacc, mybir
from concourse.bass_utils import run_bass_kernel_spmd

H = 6144
W = 6144
KH = 7
KW = 7
OH = H - KH + 1          # 6138
OW = W - KW + 1          # 6138
NCORES = 8
RPC = 768                # output rows per core (8*768 = 6144; last 6 dropped)
IRPC = RPC + KH - 1      # 774 input rows per core

# t32 scheme geometry
BK = 26                  # output rows per block (32-row tile - 6 halo)
NBLK = (RPC + BK - 1) // BK   # 30 blocks (last has 768 - 29*26 = 14 rows)
WSTEP = 4 * BK           # 104 input rows consumed per window
NWIN = (NBLK + 3) // 4   # 8 windows (last has 2 blocks)
CHW = 2048               # output columns per chunk (4 col-group quarters)
NCH = 3                  # chunks: cols [0,2048),[2048,4096),[4096,6144)
WA = CHW + 6             # 2054: window part A covers chunk 0
WB = W - CHW + 6         # 4102: part B covers chunks 1-2 (cols 2048..6144
                         # plus 6 zero-padded halo cols past the image edge)

_NC_CACHE = {}
LAST_RESULTS = None      # for the local test harness; the grader ignores this


def _build_nc_t32():
    f32 = mybir.dt.float32
    f16 = mybir.dt.float16

    nc = bacc.Bacc(trn_type="TRN2", target_bir_lowering=False, debug=False,
                   num_devices=NCORES)
    x = nc.dram_tensor("x", [IRPC, W], f16, kind="ExternalInput")
    bands = nc.dram_tensor("bands", [128, KW * 32], f16, kind="ExternalInput")
    bcol = nc.dram_tensor("bcol", [128, 1], f32, kind="ExternalInput")
    y = nc.dram_tensor("y", [RPC, W], f16, kind="ExternalOutput")

    with tile.TileContext(nc) as tc:
        with tc.tile_pool(name="const", bufs=1) as constp, \
             tc.tile_pool(name="xa", bufs=2) as xap, \
             tc.tile_pool(name="xb", bufs=2) as xbp, \
             tc.tile_pool(name="psum", bufs=8, space="PSUM") as pp, \
             tc.tile_pool(name="outs", bufs=8) as op:
            bands_sb = constp.tile([128, KW * 32], f16)
            nc.sync.dma_start(bands_sb[:], bands[:])
            bcol_sb = constp.tile([128, 1], f32)
            nc.sync.dma_start(bcol_sb[:], bcol[:])

            # Warm-up burst so the PE HAM clock-gate reaches 8/8 while the
            # first window is still loading.
            dummy = constp.tile([128, 512], f16)
            nc.vector.memset(dummy[:], 0.0)
            warm_ps = pp.tile([128, 4 * 512], f32, tag="ps4", bufs=2)
            for i in range(12):
                nc.tensor.matmul(warm_ps[:, 0:512], dummy[:, 0:128], dummy[:, 0:512],
                                 start=(i == 0), stop=(i == 11))

            # Window loads. Window w holds input rows
            # [104w + 26b, 104w + 26b + 32) in partition group b, split into
            # part A (cols [0, 2054), feeds chunk 0) and part B
            # (cols [2048, 6144), feeds chunks 1-2). All loads ride the SP
            # HWDGE ring; ACT stays DMA-free for evictions.
            xa_t = [None] * NWIN
            xb_t = [None] * NWIN
            for w in range(NWIN):
                nblk_w = min(4, NBLK - 4 * w)
                xa = xap.tile([128, WA], f16)
                xb = xbp.tile([128, WB], f16)
                nc.vector.memset(xb[:, W - CHW:WB], 0.0)
                for b in range(nblk_w):
                    r0 = WSTEP * w + BK * b
                    rn = min(32, IRPC - r0)
                    nc.sync.dma_start(xa[32 * b:32 * b + rn, :],
                                      x[r0:r0 + rn, 0:WA])
                    nc.sync.dma_start(xb[32 * b:32 * b + rn, 0:W - CHW],
                                      x[r0:r0 + rn, CHW:W])
                xa_t[w] = xa
                xb_t[w] = xb

            # Compute + evict + store, (window, chunk)-major. Round (w, c)
            # accumulates into ONE 4-bank psum tile ps4 [128, 2048]:
            # partition 32g+m, col 512b+n <-> block 4w+b, output row
            # 26(4w+b)+m, output col 2048c+512g+n. One eviction op per
            # round (DVE/ACT alternating) writes staging ot_wc [128, 2048];
            # stores stream out per col-group quarter g.
            from concourse.ap import AP
            evict_n = 0
            for w in range(NWIN):
                nblk_w = min(4, NBLK - 4 * w)
                for c in range(NCH):
                    srcx = xa_t[w] if c == 0 else xb_t[w]
                    coff = 0 if c == 0 else CHW * c - CHW
                    ps4 = pp.tile([128, 4 * 512], f32, name=f"ps4_{w}_{c}",
                                  tag="ps4", bufs=2)
                    for j in range(KW):
                        for g in range(4):
                            for b in range(nblk_w):
                                nc.tensor.matmul(
                                    ps4[32 * g:32 * g + BK,
                                        512 * b:512 * b + 512],
                                    bands_sb[32 * b:32 * b + 32,
                                             32 * j:32 * j + BK],
                                    srcx[32 * b:32 * b + 32,
                                         coff + 512 * g + j:
                                         coff + 512 * g + j + 512],
                                    start=(j == 0), stop=(j == KW - 1),
                                    tile_position=(32 * b, 32 * g),
                                    skip_group_check=True)
                    ot = op.tile([128, 4 * 512], f16, name=f"ot{w}_{c}",
                                 tag="ot")
                    if evict_n % 2 == 0:
                        nc.vector.tensor_scalar_add(ot[:], ps4[:],
                                                    bcol_sb[:])
                    else:
                        nc.scalar.activation(
                            ot[:], ps4[:],
                            mybir.ActivationFunctionType.Identity,
                            bias=bcol_sb[:])
                    evict_n += 1
                    # Stores: partition (32g+m), col (512b+n) of ot
                    # -> y[26(4w+b)+m, 2048c+512g+n]; row-halved so more
                    # SDMA engine pairs drain concurrently.
                    for g in range(4):
                        if nblk_w == 4:
                            for h0 in (0, 13):
                                src_ap = ot[32 * g + h0:32 * g + h0 + 13, :]
                                src3 = AP(src_ap.tensor, src_ap.offset,
                                          [[4 * 512, 13], [512, 4], [1, 512]])
                                dst3 = AP(y[:, :].tensor,
                                          (WSTEP * w + h0) * W
                                          + CHW * c + 512 * g,
                                          [[W, 13], [BK * W, 4], [1, 512]])
                                nc.gpsimd.dma_start(dst3, src3)
                        else:
                            for b in range(nblk_w):
                                k = 4 * w + b
                                mv = min(BK, RPC - BK * k)
                                src_ap = ot[32 * g:32 * g + mv,
                                            512 * b:512 * b + 512]
                                dst2 = AP(y[:, :].tensor,
                                          BK * k * W + CHW * c + 512 * g,
                                          [[W, mv], [1, 512]])
                                nc.gpsimd.dma_start(dst2, src_ap)
    nc.compile()
    return nc


def _build_nc_full(dtype_key: str):
    """Fallback: full-width 122-row banded matmuls (V1 scheme)."""
    f32 = mybir.dt.float32
    mm_dt = {"f32r": mybir.dt.float32r, "f16": mybir.dt.float16}[dtype_key]
    out_dt = mybir.dt.float16 if dtype_key == "f16" else f32
    BLK = 122
    NBLKF = (RPC + BLK - 1) // BLK
    NCT = (OW + 511) // 512

    nc = bacc.Bacc(trn_type="TRN2", target_bir_lowering=False, debug=False,
                   num_devices=NCORES)
    x = nc.dram_tensor("x", [IRPC, W], mm_dt, kind="ExternalInput")
    bands = nc.dram_tensor("bands", [128, KW * BLK], mm_dt,
                           kind="ExternalInput")
    bcol = nc.dram_tensor("bcol", [128, 1], f32, kind="ExternalInput")
    y = nc.dram_tensor("y", [RPC, OW], out_dt, kind="ExternalOutput")

    with tile.TileContext(nc) as tc:
        with tc.tile_pool(name="const", bufs=1) as constp, \
             tc.tile_pool(name="xin", bufs=2) as xp, \
             tc.tile_pool(name="warm", bufs=1, space="PSUM") as warmp, \
             tc.tile_pool(name="psum", bufs=7, space="PSUM") as pp, \
             tc.tile_pool(name="outs", bufs=3) as op:
            bands_mm = constp.tile([128, KW * BLK], mm_dt)
            nc.sync.dma_start(bands_mm[:], bands[:])
            bcol_sb = constp.tile([128, 1], f32)
            nc.sync.dma_start(bcol_sb[:], bcol[:])

            warm_ps = warmp.tile([BLK, 512], f32)
            for i in range(24):
                nc.tensor.matmul(warm_ps[:], bands_mm[0:128, 0:BLK],
                                 bands_mm[0:128, 0:512],
                                 start=(i == 0), stop=(i == 23))

            WAF = 6 * 512 + KW - 1
            x0a = constp.tile([128, WAF], mm_dt)
            x0b = constp.tile([128, W - WAF + KW - 1], mm_dt)
            ld0a = nc.sync.dma_start(x0a[:], x[0:128, 0:WAF])
            ld0b = nc.scalar.dma_start(x0b[:], x[0:128, WAF - KW + 1:W])

            for b in range(NBLKF):
                rb = b * BLK
                mv = min(BLK, RPC - rb)
                kv = mv + KH - 1
                if b > 0:
                    xmm = xp.tile([128, W], mm_dt)
                    ldeng = nc.sync if b % 2 == 0 else nc.scalar
                    ld = ldeng.dma_start(xmm[:kv, :], x[rb:rb + kv, :])
                    if b == 1:
                        tile.add_dep_helper(ld.ins, ld0a.ins, sync=True,
                                            reason="b1 load after b0 halves")
                        tile.add_dep_helper(ld.ins, ld0b.ins, sync=True,
                                            reason="b1 load after b0 halves")
                ot = op.tile([BLK, OW], out_dt)
                for ct in range(NCT):
                    c0 = 512 * ct
                    n = min(512, OW - c0)
                    if b == 0:
                        if ct < 6:
                            src, sc0 = x0a, c0
                        else:
                            src, sc0 = x0b, c0 - (WAF - KW + 1)
                    else:
                        src, sc0 = xmm, c0
                    ps = pp.tile([BLK, 512], f32)
                    for j in range(KW):
                        nc.tensor.matmul(
                            ps[:, :n],
                            bands_mm[:, j * BLK:j * BLK + BLK],
                            src[:, sc0 + j:sc0 + j + n],
                            start=(j == 0), stop=(j == KW - 1))
                    nc.vector.tensor_scalar_add(ot[:, c0:c0 + n], ps[:, :n],
                                                bcol_sb[0:BLK, :])
                nsub = 6
                step = (mv + nsub - 1) // nsub
                for p0 in range(0, mv, step):
                    pn = min(step, mv - p0)
                    nc.gpsimd.dma_start(y[rb + p0:rb + p0 + pn, :],
                                        ot[p0:p0 + pn, :])
    nc.compile()
    return nc


def _get_nc(dtype_key: str):
    if dtype_key not in _NC_CACHE:
        if dtype_key == "t32":
            _NC_CACHE[dtype_key] = _build_nc_t32()
        else:
            _NC_CACHE[dtype_key] = _build_nc_full(dtype_key)
    return _NC_CACHE[dtype_key]


def _build_bands_t32(weight: np.ndarray) -> np.ndarray:
    """bands[32b + k, 32j + m] = weight[k-m, j], 0 <= k-m < KH, m < 26."""
    bands = np.zeros((128, KW * 32), dtype=np.float32)
    m = np.arange(BK)
    for j in range(KW):
        for d in range(KH):
            bands[m + d, 32 * j + m] = np.float32(weight[d, j])
    for b in range(1, 4):
        bands[32 * b:32 * b + 32, :] = bands[0:32, :]
    return bands


def _build_bands_full(weight: np.ndarray) -> np.ndarray:
    BLK = 122
    bands = np.zeros((128, KW * BLK), dtype=np.float32)
    m = np.arange(BLK)
    for j in range(KW):
        for d in range(KH):
            bands[m + d, j * BLK + m] = np.float32(weight[d, j])
    return bands


def kernel(x: np.ndarray, weight: np.ndarray, bias: np.ndarray) -> np.ndarray:
    global LAST_RESULTS
    dtype_key = os.environ.get("CONV_DTYPE", "t32")
    trace = os.environ.get("CONV_TRACE", "") == "1"

    host_dt = np.float32 if dtype_key == "f32r" else np.float16
    xs = np.asarray(x, dtype=np.float32)
    assert xs.shape == (H, W), xs.shape
    wf = np.asarray(weight, dtype=np.float32)
    if dtype_key == "t32":
        bands = _build_bands_t32(wf).astype(host_dt)
    else:
        bands = _build_bands_full(wf).astype(host_dt)
    bcol = np.full((128, 1), np.float32(np.asarray(bias).reshape(-1)[0]),
                   dtype=np.float32)

    xpad = np.zeros((NCORES * RPC + KH - 1, W), dtype=host_dt)
    xpad[:H, :] = xs.astype(host_dt)
    in_maps = []
    for c in range(NCORES):
        xc = np.ascontiguousarray(xpad[c * RPC:c * RPC + IRPC, :])
        in_maps.append({"x": xc, "bands": bands, "bcol": bcol})

    nc = _get_nc(dtype_key)
    kwargs = {}
    if trace:
        kwargs = dict(trace=True, trace_cores=[0])
    res = run_bass_kernel_spmd(nc, in_maps, core_ids=list(range(NCORES)),
                               **kwargs)
    LAST_RESULTS = res
    out = np.concatenate([r["y"] for r in res.results], axis=0)[:OH, :OW]
    return np.ascontiguousarray(out.astype(np.float32))



# revision 2
# speedup vs baseline: 1.2322x; 1.2322x over previous
"""Trainium2 Bass kernel: 7x7 valid 2D cross-correlation on a 6144x6144 fp32
image, + scalar bias. Output 6138x6138 fp32.

Strategy (t32 scheme, v2 I/O path)
----------------------------------
Row-band sharding across 8 NeuronCores: core c computes output rows
[c*768, c*768+768) for all 6138 output columns. Inputs stream as fp16
(rel err ~5e-4, well inside the 2e-2 gate); output is stored fp16 and
upcast on host.

Per core the conv runs as 16 CONCURRENT 32x32-tile banded matmuls on the
PE array (4 row-groups x 4 col-groups, tile_position packing):

  - output rows are split into 26-row blocks; block k lives in SBUF
    partition group (k mod 4) of a 128-partition window tile holding
    input rows [104w + 26b, 104w + 26b + 32) (26 outputs + 6 halo).
  - stationary for tap j, block-group b: A[k, m] = W[k - m, j]
    (0 <= k-m < 7, m < 26), a [32, 26] band at partitions [32b, 32b+32).
  - col-group g of the array processes output-column quarter
    [2048c + 512g, +512); its matmul streams the same partitions with a
    j-shifted AP and writes PSUM partitions [32g, 32g+26) of bank b.
  - 7 taps accumulate per bank; 4 banks (blocks) per (window, chunk)
    round, double-buffered against eviction (8 PSUM banks total).

I/O path (v2): window loads ride the SP HWDGE ring as one [32, 6144]
row-block transfer each (fp16, full image width, single SBUF tile per
window, bufs=3).  Evictions (fp32 PSUM -> fp16 SBUF + bias) all run on
DVE.  Stores ride the ACT HWDGE ring as one 3-level-AP DMA per
col-group quarter (26 rows x 4 blocks x 512 cols).  GpSimd only does
memsets; no software-DGE DMAs anywhere (they serialize ~680ns/descr
on the engine and were the v1 bottleneck: stores fell behind the
compute cadence and drained for ~47us after the last matmul).
"""

import os

import numpy as np

import concourse.tile as tile
from concourse import bacc, mybir
from concourse.ap import AP
from concourse.bass_utils import run_bass_kernel_spmd

H = 6144
W = 6144
KH = 7
KW = 7
OH = H - KH + 1          # 6138
OW = W - KW + 1          # 6138
NCORES = 8
RPC = 768                # output rows per core (8*768 = 6144; last 6 dropped)
IRPC = RPC + KH - 1      # 774 input rows per core

# t32 scheme geometry
BK = 26                  # output rows per block (32-row tile - 6 halo)
NBLK = (RPC + BK - 1) // BK   # 30 blocks (last has 768 - 29*26 = 14 rows)
WSTEP = 4 * BK           # 104 input rows consumed per window
NWIN = (NBLK + 3) // 4   # 8 windows (last has 2 blocks)
CHW = 2048               # output columns per chunk (4 col-group quarters)
NCH = 3                  # chunks: cols [0,2048),[2048,4096),[4096,6144)
XW = W + 12              # window tile cols: 6144 + 12 zero halo (max read
                         # col is 4096+1536+6+512 = 6150)
YROWS = WSTEP * (NWIN - 1) + 2 * BK  # 780: y staging rows (stores write
                         # uniform 26-row blocks; host trims to 768)

_NC_CACHE = {}
LAST_RESULTS = None      # for the local test harness; the grader ignores this


def _build_nc_t32():
    f32 = mybir.dt.float32
    f16 = mybir.dt.float16

    nc = bacc.Bacc(trn_type="TRN2", target_bir_lowering=False, debug=False,
                   num_devices=NCORES)
    x = nc.dram_tensor("x", [IRPC, W], f16, kind="ExternalInput")
    bands = nc.dram_tensor("bands", [128, KW * 32], f16, kind="ExternalInput")
    bcol = nc.dram_tensor("bcol", [128, 1], f32, kind="ExternalInput")
    y = nc.dram_tensor("y", [YROWS, W], f16, kind="ExternalOutput")

    with tile.TileContext(nc) as tc:
        with tc.tile_pool(name="const", bufs=1) as constp, \
             tc.tile_pool(name="xw", bufs=3) as xp, \
             tc.tile_pool(name="psum", bufs=8, space="PSUM") as pp, \
             tc.tile_pool(name="outs", bufs=4) as op:
            bands_sb = constp.tile([128, KW * 32], f16)
            nc.sync.dma_start(bands_sb[:], bands[:])
            bcol_sb = constp.tile([128, 1], f32)
            nc.sync.dma_start(bcol_sb[:], bcol[:])

            # Warm-up burst so the PE HAM clock-gate reaches 8/8 while the
            # first window is still loading.
            dummy = constp.tile([128, 512], f16)
            nc.gpsimd.memset(dummy[:], 0.0)
            warm_ps = pp.tile([128, 4 * 512], f32, tag="ps4", bufs=2)
            for i in range(12):
                nc.tensor.matmul(warm_ps[:, 0:512], dummy[:, 0:128], dummy[:, 0:512],
                                 start=(i == 0), stop=(i == 11))

            # Window loads. Window w holds input rows
            # [104w + 26b, 104w + 26b + 32) in partition group b, full image
            # width plus a 12-col zero halo. One SP-ring DMA per block
            # (window 0 is split at col 2054 so chunk-0 matmuls can start
            # before the full width lands).
            xw_t = [None] * NWIN
            for w in range(NWIN):
                nblk_w = min(4, NBLK - 4 * w)
                xw = xp.tile([128, XW], f16)
                nc.gpsimd.memset(xw[:, W:XW], 0.0)
                for b in range(nblk_w):
                    r0 = WSTEP * w + BK * b
                    rn = min(32, IRPC - r0)
                    if w == 0:
                        nc.sync.dma_start(xw[32 * b:32 * b + rn, 0:CHW + 6],
                                          x[r0:r0 + rn, 0:CHW + 6])
                        nc.sync.dma_start(xw[32 * b:32 * b + rn, CHW + 6:W],
                                          x[r0:r0 + rn, CHW + 6:W])
                    else:
                        nc.sync.dma_start(xw[32 * b:32 * b + rn, 0:W],
                                          x[r0:r0 + rn, 0:W])
                xw_t[w] = xw

            # Compute + evict + store, (window, chunk)-major. Round (w, c)
            # accumulates into ONE 4-bank psum tile ps4 [128, 2048]:
            # partition 32g+m, col 512b+n <-> block 4w+b, output row
            # 26(4w+b)+m, output col 2048c+512g+n. One DVE eviction per
            # round writes staging ot_wc [128, 2048]; one ACT-ring store
            # per col-group quarter g drains it to DRAM.
            for w in range(NWIN):
                nblk_w = min(4, NBLK - 4 * w)
                for c in range(NCH):
                    srcx = xw_t[w]
                    ps4 = pp.tile([128, 4 * 512], f32, name=f"ps4_{w}_{c}",
                                  tag="ps4", bufs=2)
                    for j in range(KW):
                        for g in range(4):
                            for b in range(nblk_w):
                                nc.tensor.matmul(
                                    ps4[32 * g:32 * g + BK,
                                        512 * b:512 * b + 512],
                                    bands_sb[32 * b:32 * b + 32,
                                             32 * j:32 * j + BK],
                                    srcx[32 * b:32 * b + 32,
                                         CHW * c + 512 * g + j:
                                         CHW * c + 512 * g + j + 512],
                                    start=(j == 0), stop=(j == KW - 1),
                                    tile_position=(32 * b, 32 * g),
                                    skip_group_check=True)
                    ot = op.tile([128, 4 * 512], f16, name=f"ot{w}_{c}",
                                 tag="ot")
                    nc.vector.tensor_scalar_add(ot[:, 0:512 * nblk_w],
                                                ps4[:, 0:512 * nblk_w],
                                                bcol_sb[:])
                    # Stores: partition (32g+m), col (512b+n) of ot
                    # -> y[26(4w+b)+m, 2048c+512g+n]. One 3-level-AP DMA
                    # per col-group on the ACT HWDGE ring; walk order
                    # (m, b, n) on both sides.
                    for g in range(4):
                        src_ap = ot[32 * g:32 * g + BK, :]
                        src3 = AP(src_ap.tensor, src_ap.offset,
                                  [[4 * 512, BK], [512, nblk_w], [1, 512]])
                        dst3 = AP(y[:, :].tensor,
                                  WSTEP * w * W + CHW * c + 512 * g,
                                  [[W, BK], [BK * W, nblk_w], [1, 512]])
                        nc.scalar.dma_start(dst3, src3)
    nc.compile()
    return nc


def _get_nc(dtype_key: str):
    if dtype_key not in _NC_CACHE:
        _NC_CACHE[dtype_key] = _build_nc_t32()
    return _NC_CACHE[dtype_key]


def _build_bands_t32(weight: np.ndarray) -> np.ndarray:
    """bands[32b + k, 32j + m] = weight[k-m, j], 0 <= k-m < KH, m < 26."""
    bands = np.zeros((128, KW * 32), dtype=np.float32)
    m = np.arange(BK)
    for j in range(KW):
        for d in range(KH):
            bands[m + d, 32 * j + m] = np.float32(weight[d, j])
    for b in range(1, 4):
        bands[32 * b:32 * b + 32, :] = bands[0:32, :]
    return bands


def kernel(x: np.ndarray, weight: np.ndarray, bias: np.ndarray) -> np.ndarray:
    global LAST_RESULTS
    trace = os.environ.get("CONV_TRACE", "") == "1"

    xs = np.asarray(x, dtype=np.float32)
    assert xs.shape == (H, W), xs.shape
    wf = np.asarray(weight, dtype=np.float32)
    bands = _build_bands_t32(wf).astype(np.float16)
    bcol = np.full((128, 1), np.float32(np.asarray(bias).reshape(-1)[0]),
                   dtype=np.float32)

    xpad = np.zeros((NCORES * RPC + KH - 1, W), dtype=np.float16)
    xpad[:H, :] = xs.astype(np.float16)
    in_maps = []
    for c in range(NCORES):
        xc = np.ascontiguousarray(xpad[c * RPC:c * RPC + IRPC, :])
        in_maps.append({"x": xc, "bands": bands, "bcol": bcol})

    nc = _get_nc("t32")
    kwargs = {}
    if trace:
        kwargs = dict(trace=True, trace_cores=[0])
    res = run_bass_kernel_spmd(nc, in_maps, core_ids=list(range(NCORES)),
                               **kwargs)
    LAST_RESULTS = res
    out = np.concatenate([r["y"][:RPC] for r in res.results], axis=0)[:OH, :OW]
    return np.ascontiguousarray(out.astype(np.float32))
